# revision 27
# baseline (speedup 1.0000x reference)
"""Trainium2 Bass kernel for nn_MiddleDecoder (gnn_message_passing).

Computation (reference):
  pf = features[:, :32]; ff = features[:, 32:]
  rel = (pf @ W_nbh + b_nbh)            # [N, 48] -> viewed [N*16, 3]
  h1 = relu(concat([rel_k, ff]) @ W1 + b1)   # [N*16, 256]
  h2 = relu(h1 @ W2 + b2)                    # [N*16, 128]
  fc3 = relu(h2 @ W3 + b3)                   # [N*16, 32]
  out_points = repeat(points,16) + rel*0.25
  out_batch = repeat(batch,16)

Strategy: data-parallel over N across 8 cores. Per core, transposed-activation
MLP: activations live as [feature, point] tiles so each layer is a plain
matmul with the weight as the stationary operand. Layer-1 is computed as two
PSUM-accumulated passes (K=128 over ff, K=3 over the per-neighbor rel slice),
avoiding materializing the repeated concat. float32r matmuls (fast fp32 PE
path). Outputs are re-transposed on-chip so every HBM write is contiguous.

PE operands must sit at 32-aligned SBUF base partitions, so the per-neighbor
rel rows are reshuffled (SBUF->SBUF DMA) into 4 group tiles with each
neighbor's 3 rows at partition offsets {0,32,64,96}, and W1[0:3] is likewise
replicated at those offsets.
"""

from contextlib import ExitStack

import numpy as np

N_FULL = 50000
NB = 16
RADIUS = 0.25
SPLIT = 32
FS = 128
NCORES = 8
T = 512                 # points per tile
NT = 13                 # tiles per core
NPC = T * NT            # padded points per core (6656)

_CACHE = {}


def _build(batch_words: int, npc: int, nt: int):
    """Build the Bass module. batch_words: int32 words per batch element."""
    import concourse.bacc as bacc
    import concourse.bass as bass
    import concourse.mybir as mybir
    import concourse.tile as tile
    from concourse.masks import make_identity

    dt = mybir.dt
    f32 = dt.float32
    f32r = dt.float32r
    i32 = dt.int32
    AF = mybir.ActivationFunctionType
    ALU = mybir.AluOpType

    W = batch_words
    M = npc * NB        # output rows per core

    nc = bacc.Bacc("TRN2", target_bir_lowering=False, debug=False,
                   enable_asserts=False)

    # DRAM I/O
    feat_d = nc.dram_tensor("features", [npc, SPLIT + FS], f32, kind="ExternalInput")
    pts_d = nc.dram_tensor("points", [npc, 3], f32, kind="ExternalInput")
    bat_d = nc.dram_tensor("batchw", [npc, W], i32, kind="ExternalInput")
    Wnbh_d = nc.dram_tensor("W_nbh", [SPLIT, NB * 3], f32, kind="ExternalInput")
    bnbh_d = nc.dram_tensor("b_nbh", [NB * 3], f32, kind="ExternalInput")
    W1_d = nc.dram_tensor("W1", [FS + 3, 256], f32, kind="ExternalInput")
    b1_d = nc.dram_tensor("b1", [256], f32, kind="ExternalInput")
    W2_d = nc.dram_tensor("W2", [256, 128], f32, kind="ExternalInput")
    b2_d = nc.dram_tensor("b2", [128], f32, kind="ExternalInput")
    W3_d = nc.dram_tensor("W3", [128, 32], f32, kind="ExternalInput")
    b3_d = nc.dram_tensor("b3", [32], f32, kind="ExternalInput")
    op_d = nc.dram_tensor("out_points", [M, 3], f32, kind="ExternalOutput")
    of_d = nc.dram_tensor("out_feats", [M, 32], f32, kind="ExternalOutput")
    ob_d = nc.dram_tensor("out_batch", [M, W], i32, kind="ExternalOutput")

    CH = T // 128       # 128-row chunks per tile (4)

    with tile.TileContext(nc) as tc, ExitStack() as ctx:
        const = ctx.enter_context(tc.tile_pool(name="const", bufs=1))
        # ---- constants ----
        ident = const.tile([128, 128], f32)
        make_identity(nc, ident)
        wstage = ctx.enter_context(tc.tile_pool(name="wstage", bufs=8))
        bf16 = dt.bfloat16
        def load_cast(pool, shape, src_ap, tag, wdt):
            st = wstage.tile(shape, f32, tag=f"s_{tag}", name=f"st_{tag}")
            nc.gpsimd.dma_start(out=st, in_=src_ap)
            t = pool.tile(shape, wdt, tag=tag, name=f"t_{tag}")
            nc.vector.tensor_copy(t[:, :], st[:, :])
            return t
        def load_f32r(pool, shape, src_ap, tag):
            return load_cast(pool, shape, src_ap, tag, f32r)
        def load_bf16(pool, shape, src_ap, tag):
            return load_cast(pool, shape, src_ap, tag, bf16)
        w_nbh = load_bf16(const, [SPLIT, NB * 3], Wnbh_d[:, :], "wnbh")
        b_nbh = const.tile([NB * 3, 1], f32)
        nc.sync.dma_start(out=b_nbh, in_=bnbh_d[:][:, None])
        # b_nbh broadcast to [128,48], pre-scaled by RADIUS
        bnbh_bc = const.tile([128, NB * 3], f32)
        nc.sync.dma_start(
            out=bnbh_bc,
            in_=bass.AP(tensor=bnbh_d[:].tensor, offset=0,
                        ap=[[0, 128], [1, NB * 3]]))
        nc.vector.tensor_scalar_mul(bnbh_bc[:, :], bnbh_bc[:, :], RADIUS)
        w1a = []    # K=128 part of W1 (rows 3:131), per output half
        w1b = []    # K=3 part of W1 (rows 0:3) replicated at partitions 0/32/64/96
        b1t = []
        for mb in range(2):
            w1a.append(load_bf16(const, [128, 128],
                                 W1_d[3:131, mb * 128:(mb + 1) * 128], f"w1a{mb}"))
            wbz = const.tile([128, 128], bf16, tag=f"w1bz{mb}",
                             name=f"w1bz{mb}")
            nc.vector.memset(wbz[:, :], 0.0)
            stb = wstage.tile([3, 128], f32, tag=f"s_w1b{mb}",
                              name=f"st_w1b{mb}")
            nc.sync.dma_start(out=stb, in_=W1_d[0:3, mb * 128:(mb + 1) * 128])
            nc.vector.tensor_copy(wbz[0:3, :], stb[:, :])
            # row 3 carries b1 (relz row 3 streams constant 1.0), folding the
            # layer-1 bias into the matmul accumulation
            nc.gpsimd.dma_start(out=wbz[3:4, :],
                                in_=b1_d[mb * 128:(mb + 1) * 128][None, :])
            w1b.append(wbz)
        w2 = []
        for kb in range(2):
            w2.append(load_bf16(const, [128, 128],
                                W2_d[kb * 128:(kb + 1) * 128, :], f"w2{kb}"))
        b2t = const.tile([128, 1], f32)
        nc.sync.dma_start(out=b2t, in_=b2_d[:][:, None])
        w3 = load_bf16(const, [128, 32], W3_d[:, :], "w3")
        ones_row = const.tile([1, NB * T], bf16, name="ones_row")
        nc.vector.memset(ones_row[:, :], 1.0)
        relz = []
        for par in range(2):
            rz = const.tile([128, NB * T], bf16, tag=f"relz{par}",
                            name=f"relz{par}")
            nc.vector.memset(rz[:, :], 0.0)
            nc.gpsimd.dma_start(out=rz[3:4, :], in_=ones_row[:, :])
            relz.append(rz)
        b3bc = const.tile([128, 32], f32)
        nc.sync.dma_start(
            out=b3bc,
            in_=bass.AP(tensor=b3_d[:].tensor, offset=0,
                        ap=[[0, 128], [1, 32]]))

        # ---- pools ----
        feat_p = ctx.enter_context(tc.tile_pool(name="feat", bufs=6))
        pts_p = ctx.enter_context(tc.tile_pool(name="pts", bufs=6))
        bat_p = ctx.enter_context(tc.tile_pool(name="bat", bufs=6))
        fft_p = ctx.enter_context(tc.tile_pool(name="fft", bufs=2))
        pft_p = ctx.enter_context(tc.tile_pool(name="pft", bufs=2))
        relt_p = ctx.enter_context(tc.tile_pool(name="relt", bufs=2))
        h1_p = ctx.enter_context(tc.tile_pool(name="h1", bufs=2))
        h2_p = ctx.enter_context(tc.tile_pool(name="h2", bufs=2))
        tmp_p = ctx.enter_context(tc.tile_pool(name="tmp", bufs=3))
        stage_p = ctx.enter_context(tc.tile_pool(name="stage", bufs=2))
        opst_p = ctx.enter_context(tc.tile_pool(name="opst", bufs=6))
        batst_p = ctx.enter_context(tc.tile_pool(name="batst", bufs=6))

        ps_h1 = ctx.enter_context(tc.tile_pool(name="ps_h1", bufs=2, space="PSUM"))
        ps_h2 = ctx.enter_context(tc.tile_pool(name="ps_h2", bufs=1, space="PSUM"))
        ps_f3 = ctx.enter_context(tc.tile_pool(name="ps_f3", bufs=1, space="PSUM"))
        ps_mix = ctx.enter_context(tc.tile_pool(name="ps_mix", bufs=2, space="PSUM"))

        def r(ap):
            return ap

        for it in range(nt):
            n0 = it * T

            # ---- load + transpose inputs ----
            ffT = fft_p.tile([128, T], bf16)      # features[:,32:].T
            pfT = pft_p.tile([SPLIT, T], bf16)    # features[:,:32].T
            pts_c = []
            for c in range(CH):
                r0 = n0 + c * 128
                ft = feat_p.tile([128, SPLIT + FS], f32, tag="feat")
                nc.sync.dma_start(out=ft, in_=feat_d[r0:r0 + 128, :])
                pt = pts_p.tile([128, 3], f32, tag="pts")
                nc.sync.dma_start(out=pt, in_=pts_d[r0:r0 + 128, :])
                pts_c.append(pt)

                ptr = ps_mix.tile([128, 128], f32, tag="mix")
                nc.tensor.transpose(ptr[:, :], ft[:, SPLIT:], ident[:, :])
                nc.vector.tensor_copy(ffT[:, c * 128:(c + 1) * 128], ptr[:, :])
                ptr2 = ps_mix.tile([SPLIT, 128], f32, tag="mix")
                nc.tensor.transpose(ptr2[:, :], ft[:, 0:SPLIT], ident[:, :])
                nc.vector.tensor_copy(pfT[:, c * 128:(c + 1) * 128], ptr2[:, :])

            # ---- rel (transposed): [48, T], then reshuffle to group tiles ----
            prel = ps_mix.tile([NB * 3, T], f32, tag="mix")
            nc.tensor.matmul(prel[:, :], r(w_nbh[:, :]), r(pfT[:, :]),
                             start=True, stop=True)
            relT = relt_p.tile([NB * 3, T], bf16)
            nc.scalar.activation(relT[:, :], prel[:, :], AF.Identity,
                                 bias=b_nbh[:, :])
            # reshuffle rel rows into the zero-padded K=128 rhs tile:
            # neighbor k's 3 rel rows at rows 0:3 of column block k; rows
            # 3:128 stay zero so the L1 rel pass is a dense K=128 matmul
            # (small-K matmuls de-warm the PE clock gate).
            rel3 = relz[it % 2]
            for k in range(NB):
                nc.gpsimd.dma_start(out=rel3[0:3, k * T:(k + 1) * T],
                                    in_=relT[3 * k:3 * k + 3, :])

            # ---- output_points + output_batch per 128-chunk ----
            for c in range(CH):
                r0 = n0 + c * 128
                prn = ps_mix.tile([128, NB * 3], f32, tag="mix")
                nc.tensor.matmul(prn[:, :], r(pfT[:, c * 128:(c + 1) * 128]),
                                 r(w_nbh[:, :]), start=True, stop=True)
                ptile = pts_c[c]
                pbc = bass.AP(tensor=ptile[:, :].tensor, offset=ptile[:, :].offset,
                              ap=[ptile[:, :].ap[0], [0, NB], ptile[:, :].ap[1]])
                comb = opst_p.tile([128, NB * 3], f32, tag="comb")
                cv = comb[:, :].rearrange("p (n three) -> p n three", n=NB)
                bv3 = bnbh_bc[:, :].rearrange("p (n three) -> p n three", n=NB)
                nc.vector.tensor_add(cv, bv3, pbc)
                opst = opst_p.tile([128, NB * 3], f32, tag="opst")
                nc.vector.tensor_scalar_mul(opst[:, :], prn[:, :], RADIUS)
                nc.vector.tensor_add(opst[:, :], opst[:, :], comb[:, :])
                nc.sync.dma_start(out=op_d[r0 * NB:(r0 + 128) * NB, :],
                                  in_=opst[:, :])

                bt = bat_p.tile([128, W], i32, tag="bat")
                nc.sync.dma_start(out=bt, in_=bat_d[r0:r0 + 128, :])
                brep = batst_p.tile([128, NB * W], i32, tag="batst")
                bv = brep[:, :].rearrange("p (n w) -> p n w", n=NB)
                bbc = bass.AP(tensor=bt[:, :].tensor, offset=bt[:, :].offset,
                              ap=[bt[:, :].ap[0], [0, NB], bt[:, :].ap[1]])
                nc.gpsimd.tensor_copy(bv, bbc)
                nc.sync.dma_start(out=ob_d[r0 * NB:(r0 + 128) * NB, :],
                                  in_=brep[:, :])

            # ---- MLP over 16 neighbors ----
            stage = stage_p.tile([128, CH * NB * 32], bf16, tag="stage")
            for k in range(NB):
                ph1 = ps_h1.tile([128, 2 * T], f32, tag="h1")
                for mb in range(2):
                    half = ph1[:, mb * T:(mb + 1) * T]
                    nc.tensor.matmul(half, r(w1a[mb][:, :]), r(ffT[:, :]),
                                     start=True, stop=False)
                    nc.tensor.matmul(
                        half,
                        r(w1b[mb][:, :]),
                        r(rel3[:, k * T:(k + 1) * T]),
                        start=False, stop=True)  # K=128 (pad + bias row)
                h1t = h1_p.tile([128, 2 * T], bf16, tag="h1sb")
                nc.scalar.activation(h1t[:, :], ph1[:, :], AF.Relu, bias=0.0)
                ph2 = ps_h2.tile([128, T], f32, tag="h2")
                nc.tensor.matmul(ph2[:, :], r(w2[0][:, :]), r(h1t[:, 0:T]),
                                 start=True, stop=False)
                nc.tensor.matmul(ph2[:, :], r(w2[1][:, :]), r(h1t[:, T:2 * T]),
                                 start=False, stop=True)
                h2t = h2_p.tile([128, T], bf16, tag="h2sb")
                if k % 2 == 0:
                    nc.scalar.activation(h2t[:, :], ph2[:, :], AF.Relu,
                                         bias=b2t[:, :])
                else:
                    nc.vector.tensor_scalar(h2t[:, :], ph2[:, :], b2t[:, :],
                                            0.0, op0=ALU.add, op1=ALU.max)
                # L3 natural orientation: stationary = h2 column-chunk,
                # moving = W3 -> out [128 rows, 32] per chunk, no transposes
                pf3n = ps_f3.tile([128, CH * 32], f32, tag="f3")
                for c in range(CH):
                    nc.tensor.matmul(pf3n[:, 32 * c:32 * c + 32],
                                     r(h2t[:, c * 128:(c + 1) * 128]),
                                     r(w3[:, :]), start=True, stop=True)
                tmp = tmp_p.tile([128, CH * 32], bf16, tag="tmp")
                pv = pf3n[:, :].rearrange("p (c f) -> p c f", f=32)
                bv = bass.AP(tensor=b3bc[:, :].tensor,
                             offset=b3bc[:, :].offset,
                             ap=[b3bc[:, :].ap[0], [0, CH], b3bc[:, :].ap[1]])
                tv = tmp[:, :].rearrange("p (c f) -> p c f", f=32)
                nc.vector.tensor_add(tv, pv, bv)
                sv = bass.AP(tensor=stage[:, :].tensor,
                             offset=stage[:, :].offset + 32 * k,
                             ap=[stage[:, :].ap[0], [NB * 32, CH], [1, 32]])
                nc.vector.tensor_scalar_max(sv, tv, 0.0)
            for c in range(CH):
                r0 = n0 + c * 128
                nc.gpsimd.dma_start(
                    out=of_d[r0 * NB:(r0 + 128) * NB, :],
                    in_=stage[:, c * NB * 32:(c + 1) * NB * 32])

    nc.compile()
    return nc


def _get_nc(batch_words: int, npc: int = NPC, nt: int = NT):
    key = (batch_words, npc, nt)
    if key not in _CACHE:
        _CACHE[key] = _build(batch_words, npc, nt)
    return _CACHE[key]


def kernel(points, features, batch, W_nbh, b_nbh, W1, b1, W2, b2, W3, b3):
    from concourse.bass_utils import run_bass_kernel_spmd

    points = np.ascontiguousarray(points, np.float32)
    features = np.ascontiguousarray(features, np.float32)
    batch = np.ascontiguousarray(batch)
    n = points.shape[0]
    bdt = batch.dtype
    words = bdt.itemsize // 4
    batw = batch.view(np.int32).reshape(n, words)

    npc_raw = (n + NCORES - 1) // NCORES
    npc = ((npc_raw + T - 1) // T) * T
    nt = npc // T
    nc = _get_nc(words, npc, nt)

    weights = {
        "W_nbh": np.ascontiguousarray(W_nbh, np.float32),
        "b_nbh": np.ascontiguousarray(b_nbh, np.float32),
        "W1": np.ascontiguousarray(W1, np.float32),
        "b1": np.ascontiguousarray(b1, np.float32),
        "W2": np.ascontiguousarray(W2, np.float32),
        "b2": np.ascontiguousarray(b2, np.float32),
        "W3": np.ascontiguousarray(W3, np.float32),
        "b3": np.ascontiguousarray(b3, np.float32),
    }
    in_maps = []
    for c in range(NCORES):
        lo = c * npc_raw
        hi = min(lo + npc_raw, n)
        cnt = hi - lo
        f = np.zeros((npc, SPLIT + FS), np.float32)
        p = np.zeros((npc, 3), np.float32)
        b = np.zeros((npc, words), np.int32)
        if cnt > 0:
            f[:cnt] = features[lo:hi]
            p[:cnt] = points[lo:hi]
            b[:cnt] = batw[lo:hi]
        in_maps.append({"features": f, "points": p, "batchw": b, **weights})

    res = run_bass_kernel_spmd(nc, in_maps, core_ids=list(range(NCORES)))

    out_points = np.empty((n * NB, 3), np.float32)
    out_feats = np.empty((n * NB, 32), np.float32)
    out_batw = np.empty((n * NB, words), np.int32)
    for c in range(NCORES):
        lo = c * npc_raw
        hi = min(lo + npc_raw, n)
        cnt = (hi - lo) * NB
        if cnt <= 0:
            continue
        rr = res.results[c]
        out_points[lo * NB:hi * NB] = rr["out_points"][:cnt]
        out_feats[lo * NB:hi * NB] = rr["out_feats"][:cnt]
        out_batw[lo * NB:hi * NB] = rr["out_batch"][:cnt]

    out_batch = out_batw.copy().view(bdt).reshape(n * NB)
    return (out_points, out_feats, out_batch)


# revision 28
# speedup vs baseline: 1.0109x; 1.0109x over previous
"""Trainium2 Bass kernel for nn_MiddleDecoder (gnn_message_passing).

Computation (reference):
  pf = features[:, :32]; ff = features[:, 32:]
  rel = (pf @ W_nbh + b_nbh)            # [N, 48] -> viewed [N*16, 3]
  h1 = relu(concat([rel_k, ff]) @ W1 + b1)   # [N*16, 256]
  h2 = relu(h1 @ W2 + b2)                    # [N*16, 128]
  fc3 = relu(h2 @ W3 + b3)                   # [N*16, 32]
  out_points = repeat(points,16) + rel*0.25
  out_batch = repeat(batch,16)

Strategy: data-parallel over N across 8 cores. Per core, transposed-activation
MLP: activations live as [feature, point] tiles so each layer is a plain
matmul with the weight as the stationary operand. Layer-1 is computed as two
PSUM-accumulated passes (K=128 over ff, K=3 over the per-neighbor rel slice),
avoiding materializing the repeated concat.

Perf notes (measured on trn2):
- All matmul operands are bf16 (PSUM accumulates fp32); fp32/f32r moving
  operands stream 2-4x slower.
- Small-K matmuls interleaved with dense ones keep the PE clock gate (HAM)
  throttled at 1.2 GHz chip-wide, so the K=3 rel pass is zero-padded to a
  dense K=128 matmul: the per-neighbor rel rows live in rows 0:3 of a
  persistent [128, 16*T] tile whose remaining rows are zeros, with row 3
  streaming 1.0 so a b1 row folded into the padded W1[0:3] weight tile
  applies the layer-1 bias inside the accumulation (one fused bias-free
  [128, 2*T] relu drain).
- Layer 3 runs in natural orientation (stationary = h2 column-chunk,
  moving = W3), writing [row, fanout] blocks so no output transposes are
  needed; bias+relu happen on DVE into a bf16 staging tile that a casting
  gpsimd DMA writes to fp32 HBM.
- Outputs are staged so every HBM write is fully contiguous.
"""

from contextlib import ExitStack

import numpy as np

N_FULL = 50000
NB = 16
RADIUS = 0.25
SPLIT = 32
FS = 128
NCORES = 8
T = 512                 # points per tile
NT = 13                 # tiles per core
NPC = T * NT            # padded points per core (6656)

_CACHE = {}


def _build(batch_words: int, npc: int, nt: int):
    """Build the Bass module. batch_words: int32 words per batch element."""
    import concourse.bacc as bacc
    import concourse.bass as bass
    import concourse.mybir as mybir
    import concourse.tile as tile
    from concourse.masks import make_identity

    dt = mybir.dt
    f32 = dt.float32
    f32r = dt.float32r
    i32 = dt.int32
    AF = mybir.ActivationFunctionType
    ALU = mybir.AluOpType

    W = batch_words
    M = npc * NB        # output rows per core

    nc = bacc.Bacc("TRN2", target_bir_lowering=False, debug=False,
                   enable_asserts=False)

    # DRAM I/O
    feat_d = nc.dram_tensor("features", [npc, SPLIT + FS], f32, kind="ExternalInput")
    pts_d = nc.dram_tensor("points", [npc, 3], f32, kind="ExternalInput")
    bat_d = nc.dram_tensor("batchw", [npc, W], i32, kind="ExternalInput")
    Wnbh_d = nc.dram_tensor("W_nbh", [SPLIT, NB * 3], f32, kind="ExternalInput")
    bnbh_d = nc.dram_tensor("b_nbh", [NB * 3], f32, kind="ExternalInput")
    W1_d = nc.dram_tensor("W1", [FS + 3, 256], f32, kind="ExternalInput")
    b1_d = nc.dram_tensor("b1", [256], f32, kind="ExternalInput")
    W2_d = nc.dram_tensor("W2", [256, 128], f32, kind="ExternalInput")
    b2_d = nc.dram_tensor("b2", [128], f32, kind="ExternalInput")
    W3_d = nc.dram_tensor("W3", [128, 32], f32, kind="ExternalInput")
    b3_d = nc.dram_tensor("b3", [32], f32, kind="ExternalInput")
    op_d = nc.dram_tensor("out_points", [M, 3], f32, kind="ExternalOutput")
    of_d = nc.dram_tensor("out_feats", [M, 32], f32, kind="ExternalOutput")
    ob_d = nc.dram_tensor("out_batch", [M, W], i32, kind="ExternalOutput")

    CH = T // 128       # 128-row chunks per tile (4)

    with tile.TileContext(nc) as tc, ExitStack() as ctx:
        const = ctx.enter_context(tc.tile_pool(name="const", bufs=1))
        # ---- constants ----
        ident = const.tile([128, 128], f32)
        make_identity(nc, ident)
        wstage = ctx.enter_context(tc.tile_pool(name="wstage", bufs=8))
        bf16 = dt.bfloat16
        def load_cast(pool, shape, src_ap, tag, wdt):
            st = wstage.tile(shape, f32, tag=f"s_{tag}", name=f"st_{tag}")
            nc.gpsimd.dma_start(out=st, in_=src_ap)
            t = pool.tile(shape, wdt, tag=tag, name=f"t_{tag}")
            nc.vector.tensor_copy(t[:, :], st[:, :])
            return t
        def load_bf16(pool, shape, src_ap, tag):
            return load_cast(pool, shape, src_ap, tag, bf16)
        w_nbh = load_bf16(const, [SPLIT, NB * 3], Wnbh_d[:, :], "wnbh")
        b_nbh = const.tile([NB * 3, 1], f32)
        nc.sync.dma_start(out=b_nbh, in_=bnbh_d[:][:, None])
        # b_nbh broadcast to [128,48], pre-scaled by RADIUS
        bnbh_bc = const.tile([128, NB * 3], f32)
        nc.sync.dma_start(
            out=bnbh_bc,
            in_=bass.AP(tensor=bnbh_d[:].tensor, offset=0,
                        ap=[[0, 128], [1, NB * 3]]))
        nc.vector.tensor_scalar_mul(bnbh_bc[:, :], bnbh_bc[:, :], RADIUS)
        w1a = []    # K=128 part of W1 (rows 3:131), per output half
        w1b = []    # K=3 part of W1 (rows 0:3) replicated at partitions 0/32/64/96
        b1t = []
        for mb in range(2):
            w1a.append(load_bf16(const, [128, 128],
                                 W1_d[3:131, mb * 128:(mb + 1) * 128], f"w1a{mb}"))
            wbz = const.tile([128, 128], bf16, tag=f"w1bz{mb}",
                             name=f"w1bz{mb}")
            nc.vector.memset(wbz[:, :], 0.0)
            stb = wstage.tile([3, 128], f32, tag=f"s_w1b{mb}",
                              name=f"st_w1b{mb}")
            nc.sync.dma_start(out=stb, in_=W1_d[0:3, mb * 128:(mb + 1) * 128])
            nc.vector.tensor_copy(wbz[0:3, :], stb[:, :])
            # row 3 carries b1 (relz row 3 streams constant 1.0), folding the
            # layer-1 bias into the matmul accumulation
            nc.gpsimd.dma_start(out=wbz[3:4, :],
                                in_=b1_d[mb * 128:(mb + 1) * 128][None, :])
            w1b.append(wbz)
        w2 = []
        for kb in range(2):
            w2.append(load_bf16(const, [128, 128],
                                W2_d[kb * 128:(kb + 1) * 128, :], f"w2{kb}"))
        b2t = const.tile([128, 1], f32)
        nc.sync.dma_start(out=b2t, in_=b2_d[:][:, None])
        w3 = load_bf16(const, [128, 32], W3_d[:, :], "w3")
        ones_row = const.tile([1, NB * T], bf16, name="ones_row")
        nc.vector.memset(ones_row[:, :], 1.0)
        relz = []
        for par in range(2):
            rz = const.tile([128, NB * T], bf16, tag=f"relz{par}",
                            name=f"relz{par}")
            nc.vector.memset(rz[:, :], 0.0)
            nc.gpsimd.dma_start(out=rz[3:4, :], in_=ones_row[:, :])
            relz.append(rz)
        b3bc = const.tile([128, 32], f32)
        nc.sync.dma_start(
            out=b3bc,
            in_=bass.AP(tensor=b3_d[:].tensor, offset=0,
                        ap=[[0, 128], [1, 32]]))

        # ---- pools ----
        feat_p = ctx.enter_context(tc.tile_pool(name="feat", bufs=6))
        pts_p = ctx.enter_context(tc.tile_pool(name="pts", bufs=6))
        bat_p = ctx.enter_context(tc.tile_pool(name="bat", bufs=6))
        fft_p = ctx.enter_context(tc.tile_pool(name="fft", bufs=2))
        pft_p = ctx.enter_context(tc.tile_pool(name="pft", bufs=2))
        relt_p = ctx.enter_context(tc.tile_pool(name="relt", bufs=2))
        h1_p = ctx.enter_context(tc.tile_pool(name="h1", bufs=2))
        h2_p = ctx.enter_context(tc.tile_pool(name="h2", bufs=2))
        tmp_p = ctx.enter_context(tc.tile_pool(name="tmp", bufs=3))
        stage_p = ctx.enter_context(tc.tile_pool(name="stage", bufs=2))
        opst_p = ctx.enter_context(tc.tile_pool(name="opst", bufs=6))
        batst_p = ctx.enter_context(tc.tile_pool(name="batst", bufs=6))

        ps_h1 = ctx.enter_context(tc.tile_pool(name="ps_h1", bufs=2, space="PSUM"))
        ps_h2 = ctx.enter_context(tc.tile_pool(name="ps_h2", bufs=1, space="PSUM"))
        ps_f3 = ctx.enter_context(tc.tile_pool(name="ps_f3", bufs=1, space="PSUM"))
        ps_mix = ctx.enter_context(tc.tile_pool(name="ps_mix", bufs=2, space="PSUM"))

        def r(ap):
            return ap

        for it in range(nt):
            n0 = it * T

            # ---- load + transpose inputs ----
            ffT = fft_p.tile([128, T], bf16)      # features[:,32:].T
            pfT = pft_p.tile([SPLIT, T], bf16)    # features[:,:32].T
            pts_c = []
            for c in range(CH):
                r0 = n0 + c * 128
                ft = feat_p.tile([128, SPLIT + FS], f32, tag="feat")
                nc.sync.dma_start(out=ft, in_=feat_d[r0:r0 + 128, :])
                pt = pts_p.tile([128, 3], f32, tag="pts")
                nc.sync.dma_start(out=pt, in_=pts_d[r0:r0 + 128, :])
                pts_c.append(pt)

                ptr = ps_mix.tile([128, 128], f32, tag="mix")
                nc.tensor.transpose(ptr[:, :], ft[:, SPLIT:], ident[:, :])
                nc.vector.tensor_copy(ffT[:, c * 128:(c + 1) * 128], ptr[:, :])
                ptr2 = ps_mix.tile([SPLIT, 128], f32, tag="mix")
                nc.tensor.transpose(ptr2[:, :], ft[:, 0:SPLIT], ident[:, :])
                nc.vector.tensor_copy(pfT[:, c * 128:(c + 1) * 128], ptr2[:, :])

            # ---- rel (transposed): [48, T], then reshuffle to group tiles ----
            prel = ps_mix.tile([NB * 3, T], f32, tag="mix")
            nc.tensor.matmul(prel[:, :], r(w_nbh[:, :]), r(pfT[:, :]),
                             start=True, stop=True)
            relT = relt_p.tile([NB * 3, T], bf16)
            nc.scalar.activation(relT[:, :], prel[:, :], AF.Identity,
                                 bias=b_nbh[:, :])
            # reshuffle rel rows into the zero-padded K=128 rhs tile:
            # neighbor k's 3 rel rows at rows 0:3 of column block k; rows
            # 3:128 stay zero so the L1 rel pass is a dense K=128 matmul
            # (small-K matmuls de-warm the PE clock gate).
            rel3 = relz[it % 2]
            for k in range(NB):
                nc.gpsimd.dma_start(out=rel3[0:3, k * T:(k + 1) * T],
                                    in_=relT[3 * k:3 * k + 3, :])

            # ---- output_points + output_batch per 128-chunk ----
            for c in range(CH):
                r0 = n0 + c * 128
                prn = ps_mix.tile([128, NB * 3], f32, tag="mix")
                nc.tensor.matmul(prn[:, :], r(pfT[:, c * 128:(c + 1) * 128]),
                                 r(w_nbh[:, :]), start=True, stop=True)
                ptile = pts_c[c]
                pbc = bass.AP(tensor=ptile[:, :].tensor, offset=ptile[:, :].offset,
                              ap=[ptile[:, :].ap[0], [0, NB], ptile[:, :].ap[1]])
                comb = opst_p.tile([128, NB * 3], f32, tag="comb")
                cv = comb[:, :].rearrange("p (n three) -> p n three", n=NB)
                bv3 = bnbh_bc[:, :].rearrange("p (n three) -> p n three", n=NB)
                nc.vector.tensor_add(cv, bv3, pbc)
                opst = opst_p.tile([128, NB * 3], f32, tag="opst")
                nc.vector.tensor_scalar_mul(opst[:, :], prn[:, :], RADIUS)
                nc.vector.tensor_add(opst[:, :], opst[:, :], comb[:, :])
                nc.sync.dma_start(out=op_d[r0 * NB:(r0 + 128) * NB, :],
                                  in_=opst[:, :])

                bt = bat_p.tile([128, W], i32, tag="bat")
                nc.sync.dma_start(out=bt, in_=bat_d[r0:r0 + 128, :])
                brep = batst_p.tile([128, NB * W], i32, tag="batst")
                bv = brep[:, :].rearrange("p (n w) -> p n w", n=NB)
                bbc = bass.AP(tensor=bt[:, :].tensor, offset=bt[:, :].offset,
                              ap=[bt[:, :].ap[0], [0, NB], bt[:, :].ap[1]])
                nc.gpsimd.tensor_copy(bv, bbc)
                nc.sync.dma_start(out=ob_d[r0 * NB:(r0 + 128) * NB, :],
                                  in_=brep[:, :])

            # ---- MLP over 16 neighbors ----
            stage = stage_p.tile([128, CH * NB * 32], bf16, tag="stage")
            for k in range(NB):
                ph1 = ps_h1.tile([128, 2 * T], f32, tag="h1")
                for mb in range(2):
                    half = ph1[:, mb * T:(mb + 1) * T]
                    nc.tensor.matmul(half, r(w1a[mb][:, :]), r(ffT[:, :]),
                                     start=True, stop=False)
                    nc.tensor.matmul(
                        half,
                        r(w1b[mb][:, :]),
                        r(rel3[:, k * T:(k + 1) * T]),
                        start=False, stop=True)  # K=128 (pad + bias row)
                h1t = h1_p.tile([128, 2 * T], bf16, tag="h1sb")
                nc.scalar.activation(h1t[:, :], ph1[:, :], AF.Relu, bias=0.0)
                ph2 = ps_h2.tile([128, T], f32, tag="h2")
                nc.tensor.matmul(ph2[:, :], r(w2[0][:, :]), r(h1t[:, 0:T]),
                                 start=True, stop=False)
                nc.tensor.matmul(ph2[:, :], r(w2[1][:, :]), r(h1t[:, T:2 * T]),
                                 start=False, stop=True)
                h2t = h2_p.tile([128, T], bf16, tag="h2sb")
                if k % 2 == 0:
                    nc.scalar.activation(h2t[:, :], ph2[:, :], AF.Relu,
                                         bias=b2t[:, :])
                else:
                    nc.vector.tensor_scalar(h2t[:, :], ph2[:, :], b2t[:, :],
                                            0.0, op0=ALU.add, op1=ALU.max)
                # L3 natural orientation: stationary = h2 column-chunk,
                # moving = W3 -> out [128 rows, 32] per chunk, no transposes
                pf3n = ps_f3.tile([128, CH * 32], f32, tag="f3")
                for c in range(CH):
                    nc.tensor.matmul(pf3n[:, 32 * c:32 * c + 32],
                                     r(h2t[:, c * 128:(c + 1) * 128]),
                                     r(w3[:, :]), start=True, stop=True)
                tmp = tmp_p.tile([128, CH * 32], bf16, tag="tmp")
                pv = pf3n[:, :].rearrange("p (c f) -> p c f", f=32)
                bv = bass.AP(tensor=b3bc[:, :].tensor,
                             offset=b3bc[:, :].offset,
                             ap=[b3bc[:, :].ap[0], [0, CH], b3bc[:, :].ap[1]])
                tv = tmp[:, :].rearrange("p (c f) -> p c f", f=32)
                nc.vector.tensor_add(tv, pv, bv)
                sv = bass.AP(tensor=stage[:, :].tensor,
                             offset=stage[:, :].offset + 32 * k,
                             ap=[stage[:, :].ap[0], [NB * 32, CH], [1, 32]])
                nc.vector.tensor_scalar_max(sv, tv, 0.0)
            for c in range(CH):
                r0 = n0 + c * 128
                nc.gpsimd.dma_start(
                    out=of_d[r0 * NB:(r0 + 128) * NB, :],
                    in_=stage[:, c * NB * 32:(c + 1) * NB * 32])

    nc.compile()
    return nc


def _get_nc(batch_words: int, npc: int = NPC, nt: int = NT):
    key = (batch_words, npc, nt)
    if key not in _CACHE:
        _CACHE[key] = _build(batch_words, npc, nt)
    return _CACHE[key]


def kernel(points, features, batch, W_nbh, b_nbh, W1, b1, W2, b2, W3, b3):
    from concourse.bass_utils import run_bass_kernel_spmd

    points = np.ascontiguousarray(points, np.float32)
    features = np.ascontiguousarray(features, np.float32)
    batch = np.ascontiguousarray(batch)
    n = points.shape[0]
    bdt = batch.dtype
    words = bdt.itemsize // 4
    batw = batch.view(np.int32).reshape(n, words)

    npc_raw = (n + NCORES - 1) // NCORES
    npc = ((npc_raw + T - 1) // T) * T
    nt = npc // T
    nc = _get_nc(words, npc, nt)

    weights = {
        "W_nbh": np.ascontiguousarray(W_nbh, np.float32),
        "b_nbh": np.ascontiguousarray(b_nbh, np.float32),
        "W1": np.ascontiguousarray(W1, np.float32),
        "b1": np.ascontiguousarray(b1, np.float32),
        "W2": np.ascontiguousarray(W2, np.float32),
        "b2": np.ascontiguousarray(b2, np.float32),
        "W3": np.ascontiguousarray(W3, np.float32),
        "b3": np.ascontiguousarray(b3, np.float32),
    }
    in_maps = []
    for c in range(NCORES):
        lo = c * npc_raw
        hi = min(lo + npc_raw, n)
        cnt = hi - lo
        f = np.zeros((npc, SPLIT + FS), np.float32)
        p = np.zeros((npc, 3), np.float32)
        b = np.zeros((npc, words), np.int32)
        if cnt > 0:
            f[:cnt] = features[lo:hi]
            p[:cnt] = points[lo:hi]
            b[:cnt] = batw[lo:hi]
        in_maps.append({"features": f, "points": p, "batchw": b, **weights})

    res = run_bass_kernel_spmd(nc, in_maps, core_ids=list(range(NCORES)))

    out_points = np.empty((n * NB, 3), np.float32)
    out_feats = np.empty((n * NB, 32), np.float32)
    out_batw = np.empty((n * NB, words), np.int32)
    for c in range(NCORES):
        lo = c * npc_raw
        hi = min(lo + npc_raw, n)
        cnt = (hi - lo) * NB
        if cnt <= 0:
            continue
        rr = res.results[c]
        out_points[lo * NB:hi * NB] = rr["out_points"][:cnt]
        out_feats[lo * NB:hi * NB] = rr["out_feats"][:cnt]
        out_batw[lo * NB:hi * NB] = rr["out_batch"][:cnt]

    out_batch = out_batw.copy().view(bdt).reshape(n * NB)
    return (out_points, out_feats, out_batch)


# revision 29
# speedup vs baseline: 1.0141x; 1.0031x over previous
"""Trainium2 Bass kernel for nn_MiddleDecoder (gnn_message_passing).

Computation (reference):
  pf = features[:, :32]; ff = features[:, 32:]
  rel = (pf @ W_nbh + b_nbh)            # [N, 48] -> viewed [N*16, 3]
  h1 = relu(concat([rel_k, ff]) @ W1 + b1)   # [N*16, 256]
  h2 = relu(h1 @ W2 + b2)                    # [N*16, 128]
  fc3 = relu(h2 @ W3 + b3)                   # [N*16, 32]
  out_points = repeat(points,16) + rel*0.25
  out_batch = repeat(batch,16)

Strategy: data-parallel over N across 8 cores. Per core, transposed-activation
MLP: activations live as [feature, point] tiles so each layer is a plain
matmul with the weight as the stationary operand. Layer-1 is computed as two
PSUM-accumulated passes (K=128 over ff, K=3 over the per-neighbor rel slice),
avoiding materializing the repeated concat.

Perf notes (measured on trn2):
- All matmul operands are bf16 (PSUM accumulates fp32); fp32/f32r moving
  operands stream 2-4x slower.
- Small-K matmuls interleaved with dense ones keep the PE clock gate (HAM)
  throttled at 1.2 GHz chip-wide, so the K=3 rel pass is zero-padded to a
  dense K=128 matmul: the per-neighbor rel rows live in rows 0:3 of a
  persistent [128, 16*T] tile whose remaining rows are zeros, with row 3
  streaming 1.0 so a b1 row folded into the padded W1[0:3] weight tile
  applies the layer-1 bias inside the accumulation (one fused bias-free
  [128, 2*T] relu drain).
- Layer 3 runs in natural orientation (stationary = h2 column-chunk,
  moving = W3), writing [row, fanout] blocks so no output transposes are
  needed; bias+relu happen on DVE into a bf16 staging tile that a casting
  gpsimd DMA writes to fp32 HBM.
- Outputs are staged so every HBM write is fully contiguous.
"""

from contextlib import ExitStack

import numpy as np

N_FULL = 50000
NB = 16
RADIUS = 0.25
SPLIT = 32
FS = 128
NCORES = 8
T = 512                 # points per tile
NT = 13                 # tiles per core
NPC = T * NT            # padded points per core (6656)

_CACHE = {}


def _build(batch_words: int, npc: int, nt: int):
    """Build the Bass module. batch_words: int32 words per batch element."""
    import concourse.bacc as bacc
    import concourse.bass as bass
    import concourse.mybir as mybir
    import concourse.tile as tile
    from concourse.masks import make_identity

    dt = mybir.dt
    f32 = dt.float32
    f32r = dt.float32r
    i32 = dt.int32
    AF = mybir.ActivationFunctionType
    ALU = mybir.AluOpType

    W = batch_words
    M = npc * NB        # output rows per core

    nc = bacc.Bacc("TRN2", target_bir_lowering=False, debug=False,
                   enable_asserts=False)

    # DRAM I/O
    feat_d = nc.dram_tensor("features", [npc, SPLIT + FS], f32, kind="ExternalInput")
    pts_d = nc.dram_tensor("points", [npc, 3], f32, kind="ExternalInput")
    bat_d = nc.dram_tensor("batchw", [npc, W], i32, kind="ExternalInput")
    Wnbh_d = nc.dram_tensor("W_nbh", [SPLIT, NB * 3], f32, kind="ExternalInput")
    bnbh_d = nc.dram_tensor("b_nbh", [NB * 3], f32, kind="ExternalInput")
    W1_d = nc.dram_tensor("W1", [FS + 3, 256], f32, kind="ExternalInput")
    b1_d = nc.dram_tensor("b1", [256], f32, kind="ExternalInput")
    W2_d = nc.dram_tensor("W2", [256, 128], f32, kind="ExternalInput")
    b2_d = nc.dram_tensor("b2", [128], f32, kind="ExternalInput")
    W3_d = nc.dram_tensor("W3", [128, 32], f32, kind="ExternalInput")
    b3_d = nc.dram_tensor("b3", [32], f32, kind="ExternalInput")
    op_d = nc.dram_tensor("out_points", [M, 3], f32, kind="ExternalOutput")
    of_d = nc.dram_tensor("out_feats", [M, 32], f32, kind="ExternalOutput")
    ob_d = nc.dram_tensor("out_batch", [M, W], i32, kind="ExternalOutput")

    CH = T // 128       # 128-row chunks per tile (4)

    with tile.TileContext(nc) as tc, ExitStack() as ctx:
        const = ctx.enter_context(tc.tile_pool(name="const", bufs=1))
        # ---- constants ----
        ident = const.tile([128, 128], f32)
        make_identity(nc, ident)
        wstage = ctx.enter_context(tc.tile_pool(name="wstage", bufs=8))
        bf16 = dt.bfloat16
        def load_cast(pool, shape, src_ap, tag, wdt):
            st = wstage.tile(shape, f32, tag=f"s_{tag}", name=f"st_{tag}")
            nc.gpsimd.dma_start(out=st, in_=src_ap)
            t = pool.tile(shape, wdt, tag=tag, name=f"t_{tag}")
            nc.vector.tensor_copy(t[:, :], st[:, :])
            return t
        def load_bf16(pool, shape, src_ap, tag):
            return load_cast(pool, shape, src_ap, tag, bf16)
        w_nbh = load_bf16(const, [SPLIT, NB * 3], Wnbh_d[:, :], "wnbh")
        b_nbh = const.tile([NB * 3, 1], f32)
        nc.sync.dma_start(out=b_nbh, in_=bnbh_d[:][:, None])
        # b_nbh broadcast to [128,48], pre-scaled by RADIUS
        bnbh_bc = const.tile([128, NB * 3], f32)
        nc.sync.dma_start(
            out=bnbh_bc,
            in_=bass.AP(tensor=bnbh_d[:].tensor, offset=0,
                        ap=[[0, 128], [1, NB * 3]]))
        nc.vector.tensor_scalar_mul(bnbh_bc[:, :], bnbh_bc[:, :], RADIUS)
        w1a = []    # K=128 part of W1 (rows 3:131), per output half
        w1b = []    # K=3 part of W1 (rows 0:3) replicated at partitions 0/32/64/96
        b1t = []
        for mb in range(2):
            w1a.append(load_bf16(const, [128, 128],
                                 W1_d[3:131, mb * 128:(mb + 1) * 128], f"w1a{mb}"))
            wbz = const.tile([128, 128], bf16, tag=f"w1bz{mb}",
                             name=f"w1bz{mb}")
            nc.vector.memset(wbz[:, :], 0.0)
            stb = wstage.tile([3, 128], f32, tag=f"s_w1b{mb}",
                              name=f"st_w1b{mb}")
            nc.sync.dma_start(out=stb, in_=W1_d[0:3, mb * 128:(mb + 1) * 128])
            nc.vector.tensor_copy(wbz[0:3, :], stb[:, :])
            # row 3 carries b1 (relz row 3 streams constant 1.0), folding the
            # layer-1 bias into the matmul accumulation
            nc.gpsimd.dma_start(out=wbz[3:4, :],
                                in_=b1_d[mb * 128:(mb + 1) * 128][None, :])
            w1b.append(wbz)
        w2 = []
        for kb in range(2):
            w2.append(load_bf16(const, [128, 128],
                                W2_d[kb * 128:(kb + 1) * 128, :], f"w2{kb}"))
        b2t = const.tile([128, 1], f32)
        nc.sync.dma_start(out=b2t, in_=b2_d[:][:, None])
        w3 = load_bf16(const, [128, 32], W3_d[:, :], "w3")
        ones_row = const.tile([1, NB * T], bf16, name="ones_row")
        nc.vector.memset(ones_row[:, :], 1.0)
        relz = []
        for par in range(2):
            rz = const.tile([128, NB * T], bf16, tag=f"relz{par}",
                            name=f"relz{par}")
            nc.vector.memset(rz[:, :], 0.0)
            nc.gpsimd.dma_start(out=rz[3:4, :], in_=ones_row[:, :])
            relz.append(rz)
        b3bc = const.tile([128, 32], f32)
        nc.sync.dma_start(
            out=b3bc,
            in_=bass.AP(tensor=b3_d[:].tensor, offset=0,
                        ap=[[0, 128], [1, 32]]))

        # ---- pools ----
        feat_p = ctx.enter_context(tc.tile_pool(name="feat", bufs=6))
        pts_p = ctx.enter_context(tc.tile_pool(name="pts", bufs=6))
        bat_p = ctx.enter_context(tc.tile_pool(name="bat", bufs=6))
        fft_p = ctx.enter_context(tc.tile_pool(name="fft", bufs=3))
        pft_p = ctx.enter_context(tc.tile_pool(name="pft", bufs=3))
        relt_p = ctx.enter_context(tc.tile_pool(name="relt", bufs=3))
        h1_p = ctx.enter_context(tc.tile_pool(name="h1", bufs=2))
        h2_p = ctx.enter_context(tc.tile_pool(name="h2", bufs=2))
        tmp_p = ctx.enter_context(tc.tile_pool(name="tmp", bufs=3))
        stage_p = ctx.enter_context(tc.tile_pool(name="stage", bufs=3))
        opst_p = ctx.enter_context(tc.tile_pool(name="opst", bufs=6))
        batst_p = ctx.enter_context(tc.tile_pool(name="batst", bufs=6))

        ps_h1 = ctx.enter_context(tc.tile_pool(name="ps_h1", bufs=2, space="PSUM"))
        ps_h2 = ctx.enter_context(tc.tile_pool(name="ps_h2", bufs=1, space="PSUM"))
        ps_f3 = ctx.enter_context(tc.tile_pool(name="ps_f3", bufs=1, space="PSUM"))
        ps_mix = ctx.enter_context(tc.tile_pool(name="ps_mix", bufs=2, space="PSUM"))

        def r(ap):
            return ap

        for it in range(nt):
            n0 = it * T

            # ---- load + transpose inputs ----
            ffT = fft_p.tile([128, T], bf16)      # features[:,32:].T
            pfT = pft_p.tile([SPLIT, T], bf16)    # features[:,:32].T
            pts_c = []
            for c in range(CH):
                r0 = n0 + c * 128
                ft = feat_p.tile([128, SPLIT + FS], f32, tag="feat")
                nc.sync.dma_start(out=ft, in_=feat_d[r0:r0 + 128, :])
                pt = pts_p.tile([128, 3], f32, tag="pts")
                nc.sync.dma_start(out=pt, in_=pts_d[r0:r0 + 128, :])
                pts_c.append(pt)

                ptr = ps_mix.tile([128, 128], f32, tag="mix")
                nc.tensor.transpose(ptr[:, :], ft[:, SPLIT:], ident[:, :])
                nc.vector.tensor_copy(ffT[:, c * 128:(c + 1) * 128], ptr[:, :])
                ptr2 = ps_mix.tile([SPLIT, 128], f32, tag="mix")
                nc.tensor.transpose(ptr2[:, :], ft[:, 0:SPLIT], ident[:, :])
                nc.vector.tensor_copy(pfT[:, c * 128:(c + 1) * 128], ptr2[:, :])

            # ---- rel (transposed): [48, T], then reshuffle to group tiles ----
            prel = ps_mix.tile([NB * 3, T], f32, tag="mix")
            nc.tensor.matmul(prel[:, :], r(w_nbh[:, :]), r(pfT[:, :]),
                             start=True, stop=True)
            relT = relt_p.tile([NB * 3, T], bf16)
            nc.scalar.activation(relT[:, :], prel[:, :], AF.Identity,
                                 bias=b_nbh[:, :])
            # reshuffle rel rows into the zero-padded K=128 rhs tile:
            # neighbor k's 3 rel rows at rows 0:3 of column block k; rows
            # 3:128 stay zero so the L1 rel pass is a dense K=128 matmul
            # (small-K matmuls de-warm the PE clock gate).
            rel3 = relz[it % 2]
            for k in range(NB):
                nc.gpsimd.dma_start(out=rel3[0:3, k * T:(k + 1) * T],
                                    in_=relT[3 * k:3 * k + 3, :])

            # ---- output_points + output_batch per 128-chunk ----
            for c in range(CH):
                r0 = n0 + c * 128
                prn = ps_mix.tile([128, NB * 3], f32, tag="mix")
                nc.tensor.matmul(prn[:, :], r(pfT[:, c * 128:(c + 1) * 128]),
                                 r(w_nbh[:, :]), start=True, stop=True)
                ptile = pts_c[c]
                pbc = bass.AP(tensor=ptile[:, :].tensor, offset=ptile[:, :].offset,
                              ap=[ptile[:, :].ap[0], [0, NB], ptile[:, :].ap[1]])
                comb = opst_p.tile([128, NB * 3], f32, tag="comb")
                cv = comb[:, :].rearrange("p (n three) -> p n three", n=NB)
                bv3 = bnbh_bc[:, :].rearrange("p (n three) -> p n three", n=NB)
                nc.vector.tensor_add(cv, bv3, pbc)
                opst = opst_p.tile([128, NB * 3], f32, tag="opst")
                nc.vector.tensor_scalar_mul(opst[:, :], prn[:, :], RADIUS)
                nc.vector.tensor_add(opst[:, :], opst[:, :], comb[:, :])
                nc.sync.dma_start(out=op_d[r0 * NB:(r0 + 128) * NB, :],
                                  in_=opst[:, :])

                bt = bat_p.tile([128, W], i32, tag="bat")
                nc.sync.dma_start(out=bt, in_=bat_d[r0:r0 + 128, :])
                brep = batst_p.tile([128, NB * W], i32, tag="batst")
                bv = brep[:, :].rearrange("p (n w) -> p n w", n=NB)
                bbc = bass.AP(tensor=bt[:, :].tensor, offset=bt[:, :].offset,
                              ap=[bt[:, :].ap[0], [0, NB], bt[:, :].ap[1]])
                nc.gpsimd.tensor_copy(bv, bbc)
                nc.sync.dma_start(out=ob_d[r0 * NB:(r0 + 128) * NB, :],
                                  in_=brep[:, :])

            # ---- MLP over 16 neighbors ----
            stage = stage_p.tile([128, CH * NB * 32], bf16, tag="stage")
            for k in range(NB):
                ph1 = ps_h1.tile([128, 2 * T], f32, tag="h1")
                for mb in range(2):
                    half = ph1[:, mb * T:(mb + 1) * T]
                    nc.tensor.matmul(half, r(w1a[mb][:, :]), r(ffT[:, :]),
                                     start=True, stop=False)
                    nc.tensor.matmul(
                        half,
                        r(w1b[mb][:, :]),
                        r(rel3[:, k * T:(k + 1) * T]),
                        start=False, stop=True)  # K=128 (pad + bias row)
                h1t = h1_p.tile([128, 2 * T], bf16, tag="h1sb")
                nc.scalar.activation(h1t[:, :], ph1[:, :], AF.Relu, bias=0.0)
                ph2 = ps_h2.tile([128, T], f32, tag="h2")
                nc.tensor.matmul(ph2[:, :], r(w2[0][:, :]), r(h1t[:, 0:T]),
                                 start=True, stop=False)
                nc.tensor.matmul(ph2[:, :], r(w2[1][:, :]), r(h1t[:, T:2 * T]),
                                 start=False, stop=True)
                h2t = h2_p.tile([128, T], bf16, tag="h2sb")
                if k % 2 == 0:
                    nc.scalar.activation(h2t[:, :], ph2[:, :], AF.Relu,
                                         bias=b2t[:, :])
                else:
                    nc.vector.tensor_scalar(h2t[:, :], ph2[:, :], b2t[:, :],
                                            0.0, op0=ALU.add, op1=ALU.max)
                # L3 natural orientation: stationary = h2 column-chunk,
                # moving = W3 -> out [128 rows, 32] per chunk, no transposes
                pf3n = ps_f3.tile([128, CH * 32], f32, tag="f3")
                for c in range(CH):
                    nc.tensor.matmul(pf3n[:, 32 * c:32 * c + 32],
                                     r(h2t[:, c * 128:(c + 1) * 128]),
                                     r(w3[:, :]), start=True, stop=True)
                tmp = tmp_p.tile([128, CH * 32], bf16, tag="tmp")
                pv = pf3n[:, :].rearrange("p (c f) -> p c f", f=32)
                bv = bass.AP(tensor=b3bc[:, :].tensor,
                             offset=b3bc[:, :].offset,
                             ap=[b3bc[:, :].ap[0], [0, CH], b3bc[:, :].ap[1]])
                tv = tmp[:, :].rearrange("p (c f) -> p c f", f=32)
                nc.vector.tensor_add(tv, pv, bv)
                sv = bass.AP(tensor=stage[:, :].tensor,
                             offset=stage[:, :].offset + 32 * k,
                             ap=[stage[:, :].ap[0], [NB * 32, CH], [1, 32]])
                nc.vector.tensor_scalar_max(sv, tv, 0.0)
            for c in range(CH):
                r0 = n0 + c * 128
                nc.gpsimd.dma_start(
                    out=of_d[r0 * NB:(r0 + 128) * NB, :],
                    in_=stage[:, c * NB * 32:(c + 1) * NB * 32])

    nc.compile()
    return nc


def _get_nc(batch_words: int, npc: int = NPC, nt: int = NT):
    key = (batch_words, npc, nt)
    if key not in _CACHE:
        _CACHE[key] = _build(batch_words, npc, nt)
    return _CACHE[key]


def kernel(points, features, batch, W_nbh, b_nbh, W1, b1, W2, b2, W3, b3):
    from concourse.bass_utils import run_bass_kernel_spmd

    points = np.ascontiguousarray(points, np.float32)
    features = np.ascontiguousarray(features, np.float32)
    batch = np.ascontiguousarray(batch)
    n = points.shape[0]
    bdt = batch.dtype
    words = bdt.itemsize // 4
    batw = batch.view(np.int32).reshape(n, words)

    npc_raw = (n + NCORES - 1) // NCORES
    npc = ((npc_raw + T - 1) // T) * T
    nt = npc // T
    nc = _get_nc(words, npc, nt)

    weights = {
        "W_nbh": np.ascontiguousarray(W_nbh, np.float32),
        "b_nbh": np.ascontiguousarray(b_nbh, np.float32),
        "W1": np.ascontiguousarray(W1, np.float32),
        "b1": np.ascontiguousarray(b1, np.float32),
        "W2": np.ascontiguousarray(W2, np.float32),
        "b2": np.ascontiguousarray(b2, np.float32),
        "W3": np.ascontiguousarray(W3, np.float32),
        "b3": np.ascontiguousarray(b3, np.float32),
    }
    in_maps = []
    for c in range(NCORES):
        lo = c * npc_raw
        hi = min(lo + npc_raw, n)
        cnt = hi - lo
        f = np.zeros((npc, SPLIT + FS), np.float32)
        p = np.zeros((npc, 3), np.float32)
        b = np.zeros((npc, words), np.int32)
        if cnt > 0:
            f[:cnt] = features[lo:hi]
            p[:cnt] = points[lo:hi]
            b[:cnt] = batw[lo:hi]
        in_maps.append({"features": f, "points": p, "batchw": b, **weights})

    res = run_bass_kernel_spmd(nc, in_maps, core_ids=list(range(NCORES)))

    out_points = np.empty((n * NB, 3), np.float32)
    out_feats = np.empty((n * NB, 32), np.float32)
    out_batw = np.empty((n * NB, words), np.int32)
    for c in range(NCORES):
        lo = c * npc_raw
        hi = min(lo + npc_raw, n)
        cnt = (hi - lo) * NB
        if cnt <= 0:
            continue
        rr = res.results[c]
        out_points[lo * NB:hi * NB] = rr["out_points"][:cnt]
        out_feats[lo * NB:hi * NB] = rr["out_feats"][:cnt]
        out_batw[lo * NB:hi * NB] = rr["out_batch"][:cnt]

    out_batch = out_batw.copy().view(bdt).reshape(n * NB)
    return (out_points, out_feats, out_batch)
